# revision 1
# baseline (speedup 1.0000x reference)
"""Trainium2 Bass kernel for GroupNorm->cross-attention block (nn_Block_70325794504976).

Data-parallel over batch: 16 batches / 8 cores = 2 batches per core.
All matmuls in bf16 (PE 1 cyc/row, FWL weight loads); accumulation and the
residual path stay fp32, so final output error is damped by the small Wp
(0.02/sqrt(c)) relative to the exact fp32 residual.

Math (per batch):
  q  = (s*Wq) @ GN(x)  + s*bq          s = 1/sqrt(sqrt(d)), folded on host
  kv = Wkv' @ GN(ctx^T) + bkv'         k-half of Wkv/bkv pre-scaled by s on host
  wT[s,t] = k_h^T q_h                  (scores, [key, query] orientation)
  p = exp(wT + madd[s]) (madd = -1e9 masked, 0 else; stable-softmax max-sub
                         skipped: scores are O(5) so fp32 exp is safe)
  U = [v_h | ones*64]^T @ p            rows 64:128 = softmax denominator Z
  a_h = U[0:64] * recip(U[64:128])
  out = Wp @ a + bp + x

GroupNorm stats: per-channel sums ride free on the bf16-cast / square
activations via accum_out; a single tiny one-hot matmul folds channels into
groups.
"""

import numpy as np

NUM_HEADS = 16
C = 1024
S = 1024          # spatial 32*32
CTXD = 2048
SK = 256
D = C // NUM_HEADS          # 64
B_PER = 2                   # batches per core
NCORES = 8
EPS = 1e-5

_cache = {}


def _build_program():
    import concourse.bacc as bacc
    import concourse.tile as tile
    from concourse import mybir

    F32 = mybir.dt.float32
    BF = mybir.dt.bfloat16
    AF = mybir.ActivationFunctionType
    ALU = mybir.AluOpType
    AX = mybir.AxisListType

    nc = bacc.Bacc("TRN2", target_bir_lowering=False)

    def din(name, shape, dt=F32):
        return nc.declare_dram_parameter(name, list(shape), dt, isOutput=False)

    x_d = din("x", [B_PER, C, S])
    ctx_d = din("ctx", [B_PER, SK, CTXD])
    madd_d = din("madd", [B_PER, SK])
    wq_d = din("wqt", [C, C], BF)           # (s*Wq).T
    wkv_d = din("wkvt", [CTXD, 2 * C], BF)  # Wkv, k-half scaled, transposed
    wp_d = din("wpt", [C, C], BF)           # Wp.T
    packa_d = din("packa", [128, 212])      # ident|gx|bx|gc|bc|bqs|bkvs|bps|madd
    packb_d = din("packb", [4, 256], BF)    # bc4 | bc2
    packc_d = din("packc", [128, 6], BF)    # sel4 | sel2
    out_d = nc.declare_dram_parameter("out", [B_PER, C, S], F32, isOutput=True)

    NXC = C // 128            # 8  x channel chunks
    NCC = CTXD // 128         # 16 ctx channel chunks
    NKV = 2 * C // 128        # 16 kv output chunks
    NSC = SK // 128           # 2  key-sequence chunks
    NH = NUM_HEADS

    with tile.TileContext(nc) as tc:
        import contextlib
        est = contextlib.ExitStack()
        with est:
            consts = est.enter_context(tc.tile_pool(name="consts", bufs=1))
            big = est.enter_context(tc.tile_pool(name="big", bufs=2))       # x fp32 (cast source)
            xbp = est.enter_context(tc.tile_pool(name="xbp", bufs=16))      # x bf16
            asbp = est.enter_context(tc.tile_pool(name="asbp", bufs=16))    # a chunks bf16
            ksbp = est.enter_context(tc.tile_pool(name="ksbp", bufs=16))
            vaugp = est.enter_context(tc.tile_pool(name="vaugp", bufs=2))
            wstr = est.enter_context(tc.tile_pool(name="wstr", bufs=8))
            qsbp = est.enter_context(tc.tile_pool(name="qsbp", bufs=12))
            expwp = est.enter_context(tc.tile_pool(name="expwp", bufs=6))
            sqp = est.enter_context(tc.tile_pool(name="sqp", bufs=2))       # ctx squares
            xsqp = est.enter_context(tc.tile_pool(name="xsqp", bufs=2))     # x squares
            rzbp = est.enter_context(tc.tile_pool(name="rzbp", bufs=2))
            osbp = est.enter_context(tc.tile_pool(name="osbp", bufs=2))
            xresp = est.enter_context(tc.tile_pool(name="xresp", bufs=2))
            accp = est.enter_context(tc.tile_pool(name="accp", bufs=2))
            smallp = est.enter_context(tc.tile_pool(name="smallp", bufs=2))
            ctxrp = est.enter_context(tc.tile_pool(name="ctxrp", bufs=3))
            ctxtp = est.enter_context(tc.tile_pool(name="ctxtp", bufs=2))
            vtmpp = est.enter_context(tc.tile_pool(name="vtmpp", bufs=2))

            # PSUM: 4 single-bank slots + 2 double-bank slots = 8 banks
            ps1 = est.enter_context(tc.tile_pool(name="ps1", bufs=4, space="PSUM"))
            ps2 = est.enter_context(tc.tile_pool(name="ps2", bufs=2, space="PSUM"))

            # ---- constants: 3 packed DMAs on the DVE queue ----
            packa = consts.tile([128, 212], F32, tag="packa")
            nc.scalar.dma_start(out=packa, in_=packa_d[:, :])
            packb = consts.tile([4, 256], BF, tag="packb")
            nc.scalar.dma_start(out=packb, in_=packb_d[:, :])
            packc = consts.tile([128, 6], BF, tag="packc")
            nc.scalar.dma_start(out=packc, in_=packc_d[:, :])
            ident = packa[:, 0:128]
            gx_sb = packa[:, 128:128 + NXC]
            bx_sb = packa[:, 136:136 + NXC]
            gc_sb = packa[:, 144:144 + NCC]
            bc_sb = packa[:, 160:160 + NCC]
            bqs_sb = packa[:, 176:176 + NXC]
            bkvs_sb = packa[:, 184:184 + NKV]
            bps_sb = packa[:, 200:200 + NXC]
            madd_sb = [packa[:, 208:210], packa[:, 210:212]]
            bc4 = packb[0:4, 0:128]
            bc2 = packb[0:2, 128:256]
            sel4 = packc[:, 0:4]
            sel2 = packc[:, 4:6]

            eps_sb = consts.tile([4, 1], F32, tag="eps")
            nc.vector.memset(eps_sb, EPS)

            # ================= phase emitters ======================================
            ctxT = {}    # (b, ci) -> [128, SK] bf16 tile, normalized in place
            k_sb = {}    # (b, oc) -> [128, SK] bf16
            vaug = {}    # b -> [128, NSC, NH, 128] bf16 (cols 64:128 all ones)
            xb_all = {}  # b -> list of bf16 x chunks (normalized in place)
            xacc_all = {}
            q_all = {}   # b -> list of bf16 q chunks
            a_all = {}   # b -> list of bf16 attention-output chunks

            def phase_ctx(b):
                big_t = ctxtp.tile([128, NCC, SK], BF, tag="ctxT", name=f"ctxT_{b}")
                for ci in range(NCC):
                    ctxT[(b, ci)] = big_t[:, ci, :]
                for quart in range(4):
                    for sc in range(NSC):
                        r = ctxrp.tile([128, CTXD // 4], F32, tag="ctxraw",
                                       name=f"r_{b}_{quart}_{sc}")
                        ceng = nc.sync if (quart + sc) % 2 == 0 else nc.scalar
                        ceng.dma_start(
                            out=r,
                            in_=ctx_d[b, 128 * sc:128 * (sc + 1),
                                      quart * (CTXD // 4):(quart + 1) * (CTXD // 4)])
                        pt = ps1.tile([128, 512], F32, tag="ps1", name="ptc")
                        for cl in range(NCC // 4):
                            nc.tensor.transpose(pt[:, 128 * cl:128 * (cl + 1)],
                                                r[:, 128 * cl:128 * (cl + 1)], ident)
                        ci0 = quart * (NCC // 4)
                        nc.vector.tensor_copy(
                            out=big_t[:, ci0:ci0 + 4, 128 * sc:128 * (sc + 1)],
                            in_=pt[:, :].rearrange("p (c s) -> p c s", c=4))

                # per-channel sums (DVE reduce) + sums of squares (ACT accum_out)
                cacc = accp.tile([128, 2, NCC], F32, tag="cacc", name=f"cacc_{b}")
                for ci in range(NCC):
                    nc.vector.reduce_sum(out=cacc[:, 0, ci:ci + 1],
                                         in_=ctxT[(b, ci)], axis=AX.X)
                    sq = sqp.tile([128, SK], BF, tag="csq", name="csq")
                    nc.scalar.activation(out=sq, in_=ctxT[(b, ci)], func=AF.Square,
                                         accum_out=cacc[:, 1, ci:ci + 1])
                cacc_b = accp.tile([128, 2 * NCC], BF, tag="cacc_b", name=f"caccb_{b}")
                nc.vector.tensor_copy(out=cacc_b, in_=cacc)
                psst = ps1.tile([2, 2 * NCC], F32, tag="ps1", name="psstc")
                nc.tensor.matmul(psst, sel2, cacc_b, start=True, stop=True)

                nelem = float(64 * SK)
                stats_c = smallp.tile([2, 2, NCC], F32, tag="stats_c", name="stats_c")
                nc.vector.tensor_scalar_mul(out=stats_c, in0=psst, scalar1=1.0 / nelem)
                msq = smallp.tile([2, NCC], F32, tag="msq_c", name="msq_c")
                nc.scalar.activation(out=msq, in_=stats_c[:, 0, :], func=AF.Square)
                var = smallp.tile([2, NCC], F32, tag="var_c", name="var_c")
                nc.vector.tensor_sub(out=var, in0=stats_c[:, 1, :], in1=msq)
                sd = smallp.tile([2, NCC], F32, tag="sd_c", name="sd_c")
                nc.scalar.activation(out=sd, in_=var, func=AF.Sqrt, bias=eps_sb[0:2, :])
                rm = smallp.tile([2, 2, NCC], BF, tag="rm_c", name="rm_c")
                with nc.allow_low_precision(reason="rstd O(1), bf16 matmul input"):
                    nc.vector.reciprocal(out=rm[:, 0, :], in_=sd)
                nc.vector.tensor_copy(out=rm[:, 1, :], in_=stats_c[:, 0, :])

                psab = ps1.tile([128, 2 * NCC], F32, tag="ps1", name="psabc")
                nc.tensor.matmul(psab[:, 0:NCC], bc2, rm[:, 0, :], start=True, stop=True)
                nc.tensor.matmul(psab[:, NCC:2 * NCC], bc2, rm[:, 1, :],
                                 start=True, stop=True)
                A_c = smallp.tile([128, NCC], F32, tag="A_c", name="A_c")
                nc.vector.tensor_mul(out=A_c, in0=psab[:, 0:NCC], in1=gc_sb)
                tmp_c = smallp.tile([128, NCC], F32, tag="tmp_c", name="tmp_c")
                nc.vector.tensor_mul(out=tmp_c, in0=psab[:, NCC:2 * NCC], in1=A_c)
                B_c = smallp.tile([128, NCC], F32, tag="B_c", name="B_c")
                nc.vector.tensor_sub(out=B_c, in0=bc_sb, in1=tmp_c)
                for ci in range(NCC):
                    nc.vector.tensor_scalar(out=ctxT[(b, ci)], in0=ctxT[(b, ci)],
                                            scalar1=A_c[:, ci:ci + 1],
                                            scalar2=B_c[:, ci:ci + 1],
                                            op0=ALU.mult, op1=ALU.add)

            def phase_kv():
                for b in range(B_PER):
                    va = vaugp.tile([128, NSC, NH, 128], BF, tag="vaug", name=f"va_{b}")
                    nc.gpsimd.memset(va, 1.0)
                    vaug[b] = va
                for oc in range(NKV):
                    wv = []
                    for half in range(2):
                        wvh = wstr.tile([128, NCC // 2, 128], BF, tag="wblk",
                                        name=f"wv_{oc}_{half}")
                        eng = nc.sync
                        eng.dma_start(
                            out=wvh,
                            in_=wkv_d[1024 * half:1024 * (half + 1),
                                      128 * oc:128 * (oc + 1)].rearrange(
                                          "(j p) o -> p j o", p=128))
                        wv.append(wvh)
                    pskv = [ps1.tile([128, SK], F32, tag="ps1", name=f"pskv{b}")
                            for b in range(B_PER)]
                    for j in range(NCC):
                        for b in range(B_PER):
                            nc.tensor.matmul(pskv[b], wv[j // 8][:, j % 8, :],
                                             ctxT[(b, j)][:, :],
                                             start=(j == 0), stop=(j == NCC - 1))
                    for b in range(B_PER):
                        ps = pskv[b]
                        if oc < NXC:      # k chunk
                            kt = ksbp.tile([128, SK], BF, tag="ksb", name=f"k_{b}_{oc}")
                            nc.scalar.activation(out=kt, in_=ps, func=AF.Identity,
                                                 bias=bkvs_sb[:, oc:oc + 1], scale=1.0)
                            k_sb[(b, oc)] = kt
                        else:             # v chunk -> transpose into vaug slots
                            vc = oc - NXC
                            vt = vtmpp.tile([128, SK], F32, tag="vtmp", name="vtmp")
                            nc.scalar.activation(out=vt, in_=ps, func=AF.Identity,
                                                 bias=bkvs_sb[:, oc:oc + 1], scale=1.0)
                            for sc in range(NSC):
                                pt = ps1.tile([128, 128], F32, tag="ps1", name="ptv")
                                nc.tensor.transpose(pt, vt[:, 128 * sc:128 * (sc + 1)],
                                                    ident)
                                nc.vector.tensor_copy(
                                    out=vaug[b][:, sc, 2 * vc, 0:64], in_=pt[:, 0:64])
                                nc.vector.tensor_copy(
                                    out=vaug[b][:, sc, 2 * vc + 1, 0:64],
                                    in_=pt[:, 64:128])

            def phase_xload(b):
                xb_sb = []
                xacc = accp.tile([128, 2, NXC], F32, tag="xacc", name=f"xacc_{b}")
                xacc_all[b] = xacc
                for j in range(NXC):
                    xt = big.tile([128, S], F32, tag="big", name=f"x_{b}_{j}")
                    nc.scalar.dma_start(out=xt, in_=x_d[b, 128 * j:128 * (j + 1), :])
                    xb = xbp.tile([128, S], BF, tag="xb", name=f"xb_{b}_{j}")
                    nc.scalar.activation(out=xb, in_=xt, func=AF.Copy,
                                         accum_out=xacc[:, 0, j:j + 1])
                    xb_sb.append(xb)
                    xsq = xsqp.tile([128, S], BF, tag="xsq", name="xsq")
                    nc.scalar.activation(out=xsq, in_=xb, func=AF.Square,
                                         accum_out=xacc[:, 1, j:j + 1])
                xb_all[b] = xb_sb

            def phase_xfin(b):
                xb_sb = xb_all[b]
                xacc = xacc_all[b]
                xacc_b = accp.tile([128, 2 * NXC], BF, tag="xacc_b", name=f"xaccb_{b}")
                nc.vector.tensor_copy(out=xacc_b, in_=xacc)
                psst = ps1.tile([4, 2 * NXC], F32, tag="ps1", name="psstx")
                nc.tensor.matmul(psst, sel4, xacc_b, start=True, stop=True)

                nelem = float(32 * S)
                stats_x = smallp.tile([4, 2, NXC], F32, tag="stats_x", name="stats_x")
                nc.vector.tensor_scalar_mul(out=stats_x, in0=psst, scalar1=1.0 / nelem)
                msx = smallp.tile([4, NXC], F32, tag="msq_x", name="msq_x")
                nc.scalar.activation(out=msx, in_=stats_x[:, 0, :], func=AF.Square)
                varx = smallp.tile([4, NXC], F32, tag="var_x", name="var_x")
                nc.vector.tensor_sub(out=varx, in0=stats_x[:, 1, :], in1=msx)
                sdx = smallp.tile([4, NXC], F32, tag="sd_x", name="sd_x")
                nc.scalar.activation(out=sdx, in_=varx, func=AF.Sqrt, bias=eps_sb)
                rmx = smallp.tile([4, 2, NXC], BF, tag="rm_x", name="rm_x")
                with nc.allow_low_precision(reason="rstd O(1), bf16 matmul input"):
                    nc.vector.reciprocal(out=rmx[:, 0, :], in_=sdx)
                nc.vector.tensor_copy(out=rmx[:, 1, :], in_=stats_x[:, 0, :])

                psab = ps1.tile([128, 2 * NXC], F32, tag="ps1", name="psabx")
                nc.tensor.matmul(psab[:, 0:NXC], bc4, rmx[:, 0, :], start=True, stop=True)
                nc.tensor.matmul(psab[:, NXC:2 * NXC], bc4, rmx[:, 1, :],
                                 start=True, stop=True)
                A_x = smallp.tile([128, NXC], F32, tag="A_x", name="A_x")
                nc.vector.tensor_mul(out=A_x, in0=psab[:, 0:NXC], in1=gx_sb)
                tmp_x = smallp.tile([128, NXC], F32, tag="tmp_x", name="tmp_x")
                nc.vector.tensor_mul(out=tmp_x, in0=psab[:, NXC:2 * NXC], in1=A_x)
                B_x = smallp.tile([128, NXC], F32, tag="B_x", name="B_x")
                nc.vector.tensor_sub(out=B_x, in0=bx_sb, in1=tmp_x)
                for j in range(NXC):
                    nc.vector.tensor_scalar(out=xb_sb[j], in0=xb_sb[j],
                                            scalar1=A_x[:, j:j + 1],
                                            scalar2=B_x[:, j:j + 1],
                                            op0=ALU.mult, op1=ALU.add)

            def phase_qattn(b):
                xb_sb = xb_all[b]
                a_sb = []
                for j in range(NXC):
                    at = asbp.tile([128, S], BF, tag="asb", name=f"a_{b}_{j}")
                    a_sb.append(at)
                for hj in range(NXC):
                    wq = wstr.tile([128, NXC, 128], BF, tag="wblk", name=f"wq_{b}_{hj}")
                    nc.sync.dma_start(
                        out=wq,
                        in_=wq_d[:, 128 * hj:128 * (hj + 1)].rearrange(
                            "(j p) o -> p j o", p=128))
                    qt = qsbp.tile([128, S], BF, tag="qsb", name=f"q_{b}_{hj}")
                    for h2 in range(2):
                        sl = slice(512 * h2, 512 * (h2 + 1))
                        ps = ps1.tile([128, 512], F32, tag="ps1", name="psq")
                        for j in range(NXC):
                            nc.tensor.matmul(ps, wq[:, j, :], xb_sb[j][:, sl],
                                             start=(j == 0), stop=(j == NXC - 1))
                        nc.scalar.activation(out=qt[:, sl], in_=ps, func=AF.Identity,
                                             bias=bqs_sb[:, hj:hj + 1], scale=1.0)
                    for h in (2 * hj, 2 * hj + 1):
                        off = 64 * (h % 2)
                        expw = []
                        for sc in range(NSC):
                            psw = ps2.tile([128, S], F32, tag="ps2", name="psw")
                            for h2 in range(2):
                                sl = slice(512 * h2, 512 * (h2 + 1))
                                nc.tensor.matmul(
                                    psw[:, sl],
                                    k_sb[(b, hj)][off:off + 64, 128 * sc:128 * (sc + 1)],
                                    qt[off:off + 64, sl],
                                    start=True, stop=True)
                            ew = expwp.tile([128, S], BF, tag="expw", name="ew")
                            nc.scalar.activation(out=ew, in_=psw, func=AF.Exp,
                                                 bias=madd_sb[b][:, sc:sc + 1], scale=1.0)
                            expw.append(ew)
                        psu = ps2.tile([128, S], F32, tag="ps2", name="psu")
                        for sc in range(NSC):
                            for h2 in range(2):
                                sl = slice(512 * h2, 512 * (h2 + 1))
                                nc.tensor.matmul(psu[:, sl], vaug[b][:, sc, h, :],
                                                 expw[sc][:, sl],
                                                 start=(sc == 0), stop=(sc == NSC - 1))
                        rzb = rzbp.tile([64, S], F32, tag="rzb", name="rzb")
                        nc.vector.reciprocal(out=rzb, in_=psu[64:128, :])
                        nc.vector.tensor_mul(out=a_sb[hj][off:off + 64, :],
                                             in0=psu[0:64, :], in1=rzb)
                a_all[b] = a_sb

            def phase_p(b):
                a_sb = a_all[b]
                for o in range(NXC):
                    wp = wstr.tile([128, NXC, 128], BF, tag="wblk", name=f"wp_{b}_{o}")
                    nc.sync.dma_start(
                        out=wp,
                        in_=wp_d[:, 128 * o:128 * (o + 1)].rearrange(
                            "(j p) o -> p j o", p=128))
                    xr = xresp.tile([128, S], F32, tag="xres", name="xr")
                    nc.scalar.dma_start(out=xr, in_=x_d[b, 128 * o:128 * (o + 1), :])
                    ot = osbp.tile([128, S], F32, tag="osb", name="ot")
                    for h2 in range(2):
                        sl = slice(512 * h2, 512 * (h2 + 1))
                        ps = ps1.tile([128, 512], F32, tag="ps1", name="psp")
                        for j in range(NXC):
                            nc.tensor.matmul(ps, wp[:, j, :], a_sb[j][:, sl],
                                             start=(j == 0), stop=(j == NXC - 1))
                        nc.vector.scalar_tensor_tensor(out=ot[:, sl], in0=ps,
                                                       scalar=bps_sb[:, o:o + 1],
                                                       in1=xr[:, sl],
                                                       op0=ALU.add, op1=ALU.add)
                        nc.sync.dma_start(out=out_d[b, 128 * o:128 * (o + 1), sl],
                                          in_=ot[:, sl])

            # ================= program order ======================================
            # P(0) is deferred past qattn(1) so its matmuls fill batch-1
            # attention gaps; xnorm(b) ACT work overlaps the preceding phase.
            phase_ctx(0)
            phase_ctx(1)
            phase_xload(0)
            phase_kv()
            phase_xfin(0)
            phase_xload(1)
            phase_qattn(0)
            phase_xfin(1)
            phase_qattn(1)
            phase_p(0)
            phase_p(1)

    nc.compile()
    return nc


def _host_prep(x, context, mask, gamma_x, beta_x, gamma_c, beta_c,
               Wq, bq, Wkv, bkv, Wp, bp):
    import ml_dtypes
    f = np.float32
    bf = ml_dtypes.bfloat16
    scale = 1.0 / np.sqrt(np.sqrt(D))
    xf = np.ascontiguousarray(x.reshape(x.shape[0], C, S), dtype=f)
    ctx = np.ascontiguousarray(context, dtype=f)
    # mask==1 -> 0.0 ; mask==0 -> -1e9
    madd = np.ascontiguousarray(((mask.astype(f) - 1.0) * 1e9).astype(f))
    wqt = np.ascontiguousarray((Wq.astype(f) * scale).T.astype(bf))
    wkv_mod = np.concatenate([Wkv[:C].astype(f) * scale, Wkv[C:].astype(f)], axis=0)
    wkvt = np.ascontiguousarray(wkv_mod.T.astype(bf))
    bkv_mod = np.concatenate([bkv[:C].astype(f) * scale, bkv[C:].astype(f)], axis=0)
    wpt = np.ascontiguousarray(Wp.astype(f).T.astype(bf))

    p = np.arange(128)
    sel4 = np.zeros((128, 4), f)
    sel4[p, p // 32] = 1.0
    sel2 = np.zeros((128, 2), f)
    sel2[p, p // 64] = 1.0

    def chunked(v, n):      # [128*n] -> [128, n] with col o = v[128*o + p]
        return v.astype(f).reshape(n, 128).T

    packa = np.zeros((128, 212), f)
    packa[:, 0:128] = np.eye(128, dtype=f)
    packa[:, 128:136] = chunked(gamma_x, 8)
    packa[:, 136:144] = chunked(beta_x, 8)
    packa[:, 144:160] = chunked(gamma_c, 16)
    packa[:, 160:176] = chunked(beta_c, 16)
    packa[:, 176:184] = chunked(bq.astype(f) * scale, 8)
    packa[:, 184:200] = chunked(bkv_mod, 16)
    packa[:, 200:208] = chunked(bp, 8)
    packb = np.zeros((4, 256), f)
    packb[0:4, 0:128] = sel4.T
    packb[0:2, 128:256] = sel2.T
    packc = np.concatenate([sel4, sel2], axis=1)

    shared = {
        "wqt": wqt, "wkvt": wkvt, "wpt": wpt,
        "packb": np.ascontiguousarray(packb.astype(bf)),
        "packc": np.ascontiguousarray(packc.astype(bf)),
    }
    in_maps = []
    for c in range(NCORES):
        sl = slice(B_PER * c, B_PER * (c + 1))
        m = dict(shared)
        m["x"] = np.ascontiguousarray(xf[sl])
        m["ctx"] = np.ascontiguousarray(ctx[sl])
        m["madd"] = np.ascontiguousarray(madd[sl])
        pa = packa.copy()
        pa[:, 208:210] = madd[sl.start + 0].reshape(2, 128).T
        pa[:, 210:212] = madd[sl.start + 1].reshape(2, 128).T
        m["packa"] = pa
        in_maps.append(m)
    return in_maps


def kernel(x, context, mask, gamma_x, beta_x, gamma_c, beta_c,
           Wq, bq, Wkv, bkv, Wp, bp):
    from concourse.bass_utils import run_bass_kernel_spmd

    if "nc" not in _cache:
        _cache["nc"] = _build_program()
    nc = _cache["nc"]
    in_maps = _host_prep(x, context, mask, gamma_x, beta_x, gamma_c, beta_c,
                         Wq, bq, Wkv, bkv, Wp, bp)
    res = run_bass_kernel_spmd(nc, in_maps, list(range(NCORES)))
    outs = [res.results[c]["out"] for c in range(NCORES)]
    full = np.concatenate(outs, axis=0)          # [16, C, S]
    b, c = x.shape[0], x.shape[1]
    return full.reshape(b, c, *x.shape[2:]).astype(np.float32)

